# Initial kernel scaffold
#
"""Self-contained Trainium2 kernel for nn_EncoderSAGE (2-layer GraphSAGE,
mean aggregation), distributed over 8 NeuronCores.

Strategy (graph/data parallel, per the sharding hint):
- Nodes are sharded by id across the 8 cores (12500 dst nodes per core); the
  small weight matrices are replicated; x is replicated so every core can
  gather arbitrary source rows.
- Per core, every owned dst node gets one output "slot". Nodes are bucketed
  by padded degree D = 4*ceil(deg/4); a "tile" is 128 gathered edge-rows
  covering S = floor(128/D) slots of one bucket. Bucket capacities are padded
  to the max across cores, so all 8 cores run the same program (SPMD) with
  different data (gather indices + masks).
- Edge features are gathered with indirect DMA; the segment-mean is computed
  on the tensor engine as  PSUM[feat, S] = gathered_tile.T @ mask  where the
  host-built mask holds 1/max(deg,1) at real-edge lanes (mean fused into the
  one-hot). Aggregates accumulate transposed (feat on partitions) so the
  downstream  agg @ W_l + x_own @ W_r  matmuls need no transposes.
- Between the two layers, per-core hidden rows are exchanged with an 8-core
  AllGather (halo exchange of the full boundary set); layer-2 gather indices
  are pre-remapped by the host to (owner_core * S_pad + slot).
- The host un-permutes output rows (slot -> node id) at the end.
"""

import sys

sys.path.insert(0, "/opt/trn_rl_repo")

import numpy as np

N_NODES = 100000
N_EDGES = 1600000
C = 8
P = 128
BANK = 512
IN_C = 128
HID = 64
K_CHUNK = 32


def _round_up(a, m):
    return (a + m - 1) // m * m


class _Plan:
    pass


def build_plan(edge_index, N, C=8):
    src = np.asarray(edge_index[0], dtype=np.int64)
    dst = np.asarray(edge_index[1], dtype=np.int64)
    E = src.shape[0]
    assert N % C == 0
    NPC = N // C

    deg = np.bincount(dst, minlength=N).astype(np.int64)
    assert int(deg.max()) <= 128, f"max degree {deg.max()} > 128 unsupported"
    D_node = 4 * ((np.maximum(deg, 1) + 3) // 4)

    core_of = np.arange(N) // NPC

    Ds = np.arange(4, 132, 4)
    cnt = np.zeros((C, len(Ds)), dtype=np.int64)
    for c in range(C):
        dn = D_node[c * NPC:(c + 1) * NPC]
        cnt[c] = np.bincount(dn // 4 - 1, minlength=len(Ds))
    cap = cnt.max(axis=0)

    tiles_D, tiles_off, tiles_S = [], [], []
    slot_cursor = 0
    bucket_slots = {}
    for Di, D in enumerate(Ds):
        n = int(cap[Di])
        if n == 0:
            continue
        S_D = P // D
        slots_here = []
        while n > 0:
            S = min(S_D, n)
            bank_rem = BANK - (slot_cursor % BANK)
            if S > bank_rem:
                slot_cursor += bank_rem
            tiles_D.append(D)
            tiles_off.append(slot_cursor)
            tiles_S.append(S)
            slots_here.extend(range(slot_cursor, slot_cursor + S))
            slot_cursor += S
            n -= S
        bucket_slots[int(D)] = np.array(slots_here, dtype=np.int64)
    S_pad = _round_up(slot_cursor, BANK)
    T = len(tiles_D)
    tiles_D = np.array(tiles_D, dtype=np.int64)
    tiles_off = np.array(tiles_off, dtype=np.int64)
    tiles_S = np.array(tiles_S, dtype=np.int64)

    tile_of_slot = np.full(S_pad, -1, dtype=np.int64)
    lidx_of_slot = np.full(S_pad, -1, dtype=np.int64)
    for t in range(T):
        o, S = tiles_off[t], tiles_S[t]
        tile_of_slot[o:o + S] = t
        lidx_of_slot[o:o + S] = np.arange(S)

    slot_of = np.full(N, -1, dtype=np.int64)
    for c in range(C):
        lo, hi = c * NPC, (c + 1) * NPC
        dn = D_node[lo:hi]
        for D in np.unique(dn):
            nodes = lo + np.nonzero(dn == D)[0]
            slots = bucket_slots[int(D)][: len(nodes)]
            slot_of[nodes] = slots
    assert (slot_of >= 0).all()

    node_of_slot = np.full((C, S_pad), -1, dtype=np.int64)
    node_of_slot[core_of, slot_of] = np.arange(N)

    order = np.argsort(dst, kind="stable")
    dst_s = dst[order]
    src_s = src[order]
    group_start = np.zeros(N + 1, dtype=np.int64)
    np.cumsum(np.bincount(dst_s, minlength=N), out=group_start[1:])
    rank = np.arange(E) - group_start[dst_s]
    e_core = core_of[dst_s]
    e_slot = slot_of[dst_s]
    e_tile = tile_of_slot[e_slot]
    e_part = lidx_of_slot[e_slot] * D_node[dst_s] + rank
    assert (e_part < P).all()

    ZROW1 = N
    idx_l1 = np.full((C, P, T), ZROW1, dtype=np.int32)
    idx_l1[e_core, e_part, e_tile] = src_s.astype(np.int32)

    ZROW2 = C * S_pad
    idx_l2 = np.full((C, P, T), ZROW2, dtype=np.int32)
    h_row = core_of[src_s] * S_pad + slot_of[src_s]
    idx_l2[e_core, e_part, e_tile] = h_row.astype(np.int32)

    masks = np.zeros((C, P, S_pad), dtype=np.float32)
    inv = (1.0 / np.maximum(deg, 1.0)).astype(np.float32)
    masks[e_core, e_part, e_slot] = inv[dst_s]

    NB = S_pad // P
    d1 = node_of_slot.reshape(C, NB, P).transpose(0, 2, 1).copy()
    idx_d1 = np.where(d1 >= 0, d1, ZROW1).astype(np.int32)

    pl = _Plan()
    pl.N, pl.E, pl.C, pl.NPC = N, E, C, NPC
    pl.S_pad, pl.T, pl.NB = S_pad, T, NB
    pl.tiles_D, pl.tiles_off, pl.tiles_S = tiles_D, tiles_off, tiles_S
    pl.idx_l1, pl.idx_l2, pl.idx_d1 = idx_l1, idx_l2, idx_d1
    pl.masks = masks
    pl.slot_of, pl.core_of = slot_of, core_of
    pl.deg = deg
    return pl


class Meta:
    def __init__(self, pl, K=K_CHUNK):
        self.N = pl.N
        self.C = pl.C
        self.S_pad = pl.S_pad
        self.T = pl.T
        self.NB = pl.NB
        self.NBANK = pl.S_pad // BANK
        self.K = K
        self.tiles = list(zip(pl.tiles_D.tolist(), pl.tiles_off.tolist(),
                              pl.tiles_S.tolist()))
        self.bank_tiles = [[] for _ in range(self.NBANK)]
        for t, (D, off, S) in enumerate(self.tiles):
            self.bank_tiles[off // BANK].append(t)
        self.bank_dead = []
        for b in range(self.NBANK):
            covered = np.zeros(BANK, bool)
            for t in self.bank_tiles[b]:
                D, off, S = self.tiles[t]
                covered[off - b * BANK: off - b * BANK + S] = True
            ranges = []
            i = 0
            while i < BANK:
                if not covered[i]:
                    j = i
                    while j < BANK and not covered[j]:
                        j += 1
                    ranges.append((i, j - i))
                    i = j
                else:
                    i += 1
            self.bank_dead.append(ranges)


def build_nc(meta):
    import concourse.bacc as bacc
    import concourse.mybir as mybir
    import concourse.tile as tile
    from concourse.bass import IndirectOffsetOnAxis

    F32 = mybir.dt.float32
    I32 = mybir.dt.int32

    nc = bacc.Bacc("TRN2", target_bir_lowering=False, debug=False,
                   num_devices=meta.C)
    N, S_pad, T, NB, NBANK, K = (meta.N, meta.S_pad, meta.T, meta.NB,
                                 meta.NBANK, meta.K)
    CS = meta.C * S_pad

    x_aug = nc.dram_tensor("x_aug", [N + 1, IN_C], F32, kind="ExternalInput")
    idx_l1 = nc.dram_tensor("idx_l1", [P, T], I32, kind="ExternalInput")
    idx_l2 = nc.dram_tensor("idx_l2", [P, T], I32, kind="ExternalInput")
    idx_d1 = nc.dram_tensor("idx_d1", [P, NB], I32, kind="ExternalInput")
    masks = nc.dram_tensor("masks", [P, S_pad], F32, kind="ExternalInput")
    Wl1 = nc.dram_tensor("Wl1", [IN_C, HID], F32, kind="ExternalInput")
    Wr1 = nc.dram_tensor("Wr1", [IN_C, HID], F32, kind="ExternalInput")
    Wl2 = nc.dram_tensor("Wl2", [HID, HID], F32, kind="ExternalInput")
    Wr2 = nc.dram_tensor("Wr2", [HID, HID], F32, kind="ExternalInput")
    bias1 = nc.dram_tensor("bias1", [HID, 1], F32, kind="ExternalInput")
    bias2 = nc.dram_tensor("bias2", [HID, 1], F32, kind="ExternalInput")
    ident = nc.dram_tensor("ident", [P, P], F32, kind="ExternalInput")
    out_d = nc.dram_tensor("out", [S_pad, HID], F32, kind="ExternalOutput")

    h_own = nc.dram_tensor("h_own", [S_pad, HID], F32)
    h_table = nc.dram_tensor("h_table", [CS + P, HID], F32, addr_space="Shared")

    with tile.TileContext(nc) as tc:
        with (
            tc.tile_pool(name="persist", bufs=1) as pp,
            tc.tile_pool(name="gather", bufs=2) as gp,
            tc.tile_pool(name="small", bufs=2) as sp,
            tc.tile_pool(name="psum", bufs=2, space="PSUM") as psp,
        ):
            idx1_sb = pp.tile([P, T], I32, tag="idx1")
            idx2_sb = pp.tile([P, T], I32, tag="idx2")
            idxd1_sb = pp.tile([P, NB], I32, tag="idxd1")
            wl1_sb = pp.tile([IN_C, HID], F32, tag="wl1")
            wr1_sb = pp.tile([IN_C, HID], F32, tag="wr1")
            wl2_sb = pp.tile([HID, HID], F32, tag="wl2")
            wr2_sb = pp.tile([HID, HID], F32, tag="wr2")
            b1_sb = pp.tile([HID, 1], F32, tag="b1")
            b2_sb = pp.tile([HID, 1], F32, tag="b2")
            id_sb = pp.tile([P, P], F32, tag="ident")
            hT_full = pp.tile([HID, S_pad], F32, tag="hT")
            h_stage = pp.tile([P, NB * HID], F32, tag="hstage")
            zero_sb = pp.tile([P, HID], F32, tag="zero")

            nc.sync.dma_start(out=idx1_sb[:], in_=idx_l1[:])
            nc.sync.dma_start(out=idx2_sb[:], in_=idx_l2[:])
            nc.sync.dma_start(out=idxd1_sb[:], in_=idx_d1[:])
            nc.sync.dma_start(out=wl1_sb[:], in_=Wl1[:])
            nc.sync.dma_start(out=wr1_sb[:], in_=Wr1[:])
            nc.sync.dma_start(out=wl2_sb[:], in_=Wl2[:])
            nc.sync.dma_start(out=wr2_sb[:], in_=Wr2[:])
            nc.sync.dma_start(out=b1_sb[:], in_=bias1[:])
            nc.sync.dma_start(out=b2_sb[:], in_=bias2[:])
            nc.sync.dma_start(out=id_sb[:], in_=ident[:])
            nc.vector.memset(zero_sb[:], 0.0)
            nc.sync.dma_start(out=h_table[CS:CS + P, :], in_=zero_sb[:])

            def layer(layer_i):
                first = layer_i == 0
                F = IN_C if first else HID
                idx_sb = idx1_sb if first else idx2_sb
                table = x_aug if first else h_table
                wl_sb, wr_sb = (wl1_sb, wr1_sb) if first else (wl2_sb, wr2_sb)
                b_sb = b1_sb if first else b2_sb
                stage = h_stage

                gbufs = {}

                def get_gbuf(c):
                    if c not in gbufs:
                        nk = min(K, T - c * K)
                        g = gp.tile([P, K * F], F32, tag=f"gbuf{layer_i}")
                        nc.gpsimd.indirect_dma_start(
                            out=g[:, :nk * F],
                            out_offset=None,
                            in_=table[:],
                            in_offset=IndirectOffsetOnAxis(
                                ap=idx_sb[:, c * K:c * K + nk], axis=0),
                        )
                        gbufs[c] = g
                    return gbufs[c]

                for b in range(NBANK):
                    mask_sb = sp.tile([P, BANK], F32, tag="mask")
                    nc.sync.dma_start(out=mask_sb[:],
                                      in_=masks[:, b * BANK:(b + 1) * BANK])
                    ps_seg = psp.tile([P, BANK], F32, tag="seg")
                    for (o, ln) in meta.bank_dead[b]:
                        nc.vector.memset(ps_seg[:F, o:o + ln], 0.0)
                    for t in meta.bank_tiles[b]:
                        D, off, S = meta.tiles[t]
                        o = off - b * BANK
                        g = get_gbuf(t // K)
                        j = t % K
                        nc.tensor.matmul(
                            out=ps_seg[:F, o:o + S],
                            lhsT=g[:, j * F:(j + 1) * F],
                            rhs=mask_sb[:, o:o + S],
                            start=True, stop=True,
                        )
                    aggT = sp.tile([P, BANK], F32, tag="aggT")
                    nc.vector.tensor_copy(out=aggT[:F], in_=ps_seg[:F])

                    if first:
                        ownT = sp.tile([P, BANK], F32, tag="ownT")
                        og = sp.tile([P, 4 * IN_C], F32, tag="og")
                        nc.gpsimd.indirect_dma_start(
                            out=og[:],
                            out_offset=None,
                            in_=table[:],
                            in_offset=IndirectOffsetOnAxis(
                                ap=idxd1_sb[:, b * 4:(b + 1) * 4], axis=0),
                        )
                        for blk in range(4):
                            ps_t = psp.tile([P, P], F32, tag="tp")
                            nc.tensor.transpose(
                                out=ps_t[:IN_C, :P],
                                in_=og[:, blk * IN_C:(blk + 1) * IN_C],
                                identity=id_sb[:],
                            )
                            nc.vector.tensor_copy(
                                out=ownT[:F, blk * P:(blk + 1) * P],
                                in_=ps_t[:F, :P])
                        own_rhs = ownT[:F, :]
                    else:
                        own_rhs = hT_full[:HID, b * BANK:(b + 1) * BANK]

                    ps_d = psp.tile([HID, BANK], F32, tag="down")
                    nc.tensor.matmul(out=ps_d[:], lhsT=wl_sb[:F, :],
                                     rhs=aggT[:F, :], start=True, stop=False)
                    nc.tensor.matmul(out=ps_d[:], lhsT=wr_sb[:F, :],
                                     rhs=own_rhs, start=False, stop=True)

                    hT_bank = sp.tile([HID, BANK], F32, tag="hTb")
                    if first:
                        nc.vector.tensor_scalar(
                            out=hT_bank[:], in0=ps_d[:], scalar1=b_sb[:, :1],
                            scalar2=0.0, op0=mybir.AluOpType.add,
                            op1=mybir.AluOpType.max)
                        nc.vector.tensor_copy(
                            out=hT_full[:, b * BANK:(b + 1) * BANK],
                            in_=hT_bank[:])
                    else:
                        nc.vector.tensor_scalar(
                            out=hT_bank[:], in0=ps_d[:], scalar1=b_sb[:, :1],
                            scalar2=None, op0=mybir.AluOpType.add)

                    for blk in range(4):
                        B = b * 4 + blk
                        ps_t2 = psp.tile([P, P], F32, tag="tp")
                        nc.tensor.transpose(
                            out=ps_t2[:P, :HID],
                            in_=hT_bank[:, blk * P:(blk + 1) * P],
                            identity=id_sb[:HID, :HID],
                        )
                        nc.vector.tensor_copy(
                            out=stage[:, B * HID:(B + 1) * HID],
                            in_=ps_t2[:P, :HID])

                dst = (h_own if first else out_d)
                nc.sync.dma_start(
                    out=dst[:].rearrange("(nb p) f -> p nb f", p=P),
                    in_=stage[:].rearrange("p (nb f) -> p nb f", f=HID),
                )
                if first:
                    nc.gpsimd.collective_compute(
                        "AllGather",
                        mybir.AluOpType.bypass,
                        replica_groups=[list(range(meta.C))],
                        ins=[h_own[:]],
                        outs=[h_table[0:CS, :]],
                    )

            layer(0)
            layer(1)

    nc.compile()
    return nc


def _in_maps(pl, x, W_l1, b_l1, W_r1, W_l2, b_l2, W_r2):
    x_aug = np.concatenate([x, np.zeros((1, IN_C), np.float32)], 0)
    x_aug = np.ascontiguousarray(x_aug)
    ident = np.eye(P, dtype=np.float32)
    maps = []
    for c in range(pl.C):
        maps.append({
            "x_aug": x_aug,
            "idx_l1": np.ascontiguousarray(pl.idx_l1[c]),
            "idx_l2": np.ascontiguousarray(pl.idx_l2[c]),
            "idx_d1": np.ascontiguousarray(pl.idx_d1[c]),
            "masks": np.ascontiguousarray(pl.masks[c]),
            "Wl1": np.ascontiguousarray(W_l1.astype(np.float32)),
            "Wr1": np.ascontiguousarray(W_r1.astype(np.float32)),
            "Wl2": np.ascontiguousarray(W_l2.astype(np.float32)),
            "Wr2": np.ascontiguousarray(W_r2.astype(np.float32)),
            "bias1": np.ascontiguousarray(
                np.asarray(b_l1, np.float32).reshape(HID, 1)),
            "bias2": np.ascontiguousarray(
                np.asarray(b_l2, np.float32).reshape(HID, 1)),
            "ident": ident,
        })
    return maps


_CACHE = {}


def kernel(x, edge_index, W_l1, b_l1, W_r1, W_l2, b_l2, W_r2,
           trace=False, _return_results=False):
    x = np.asarray(x, dtype=np.float32)
    edge_index = np.asarray(edge_index)
    N = x.shape[0]

    pl = build_plan(edge_index, N, C)
    meta = Meta(pl, K=K_CHUNK)

    # compile cache keyed by the compile-time structure
    key = (N, pl.S_pad, pl.T, tuple(pl.tiles_D.tolist()),
           tuple(pl.tiles_off.tolist()), tuple(pl.tiles_S.tolist()))
    if key not in _CACHE:
        _CACHE.clear()
        _CACHE[key] = build_nc(meta)
    nc = _CACHE[key]

    maps = _in_maps(pl, x, W_l1, b_l1, W_r1, W_l2, b_l2, W_r2)

    from concourse.bass_utils import run_bass_kernel_spmd
    res = run_bass_kernel_spmd(nc, maps, core_ids=list(range(C)),
                               trace=trace, trace_cores=[0] if trace else None)

    out = np.zeros((N, HID), np.float32)
    for c in range(C):
        r = res.results[c]["out"]
        nodes = np.nonzero(pl.core_of == c)[0]
        out[nodes] = r[pl.slot_of[nodes]]
    if _return_results:
        return out, res
    return out


# revision 1
# speedup vs baseline: 30.2030x; 30.2030x over previous
"""Self-contained Trainium2 kernel for nn_EncoderSAGE (2-layer GraphSAGE,
mean aggregation), distributed over 8 NeuronCores.

Strategy (graph/data parallel, per the sharding hint):
- Nodes are sharded by id across the 8 cores (12500 dst nodes per core); the
  small weight matrices are replicated; x is replicated so every core can
  gather arbitrary source rows.
- Per core, every owned dst node gets one output "slot". Nodes are bucketed
  by padded degree D = 4*ceil(deg/4); a "tile" is 128 gathered edge-rows
  covering S = floor(128/D) slots of one bucket. Bucket capacities are padded
  to the max across cores, so all 8 cores run the same program (SPMD) with
  different data (gather indices + masks).
- Edge features are gathered with indirect DMA; the segment-mean is computed
  on the tensor engine as  PSUM[feat, S] = gathered_tile.T @ mask  where the
  host-built mask holds 1/max(deg,1) at real-edge lanes (mean fused into the
  one-hot). Aggregates accumulate transposed (feat on partitions) so the
  downstream  agg @ W_l + x_own @ W_r  matmuls need no transposes.
- Between the two layers, per-core hidden rows are exchanged with an 8-core
  AllGather (halo exchange of the full boundary set); layer-2 gather indices
  are pre-remapped by the host to (owner_core * S_pad + slot).
- The host un-permutes output rows (slot -> node id) at the end.
"""

import sys

sys.path.insert(0, "/opt/trn_rl_repo")

import numpy as np

N_NODES = 100000
N_EDGES = 1600000
C = 8
P = 128
BANK = 512
IN_C = 128
HID = 64
K_CHUNK = 32


def _round_up(a, m):
    return (a + m - 1) // m * m


class _Plan:
    pass


def build_plan(edge_index, N, C=8):
    src = np.asarray(edge_index[0], dtype=np.int64)
    dst = np.asarray(edge_index[1], dtype=np.int64)
    E = src.shape[0]
    assert N % C == 0
    NPC = N // C

    deg = np.bincount(dst, minlength=N).astype(np.int64)
    assert int(deg.max()) <= 128, f"max degree {deg.max()} > 128 unsupported"
    D_node = 4 * ((np.maximum(deg, 1) + 3) // 4)

    core_of = np.arange(N) // NPC

    Ds = np.arange(4, 132, 4)
    cnt = np.zeros((C, len(Ds)), dtype=np.int64)
    for c in range(C):
        dn = D_node[c * NPC:(c + 1) * NPC]
        cnt[c] = np.bincount(dn // 4 - 1, minlength=len(Ds))
    cap = cnt.max(axis=0)

    tiles_D, tiles_off, tiles_S = [], [], []
    slot_cursor = 0
    bucket_slots = {}
    for Di, D in enumerate(Ds):
        n = int(cap[Di])
        if n == 0:
            continue
        S_D = P // D
        slots_here = []
        while n > 0:
            S = min(S_D, n)
            bank_rem = BANK - (slot_cursor % BANK)
            if S > bank_rem:
                slot_cursor += bank_rem
            tiles_D.append(D)
            tiles_off.append(slot_cursor)
            tiles_S.append(S)
            slots_here.extend(range(slot_cursor, slot_cursor + S))
            slot_cursor += S
            n -= S
        bucket_slots[int(D)] = np.array(slots_here, dtype=np.int64)
    S_pad = _round_up(slot_cursor, BANK)
    T = len(tiles_D)
    tiles_D = np.array(tiles_D, dtype=np.int64)
    tiles_off = np.array(tiles_off, dtype=np.int64)
    tiles_S = np.array(tiles_S, dtype=np.int64)

    tile_of_slot = np.full(S_pad, -1, dtype=np.int64)
    lidx_of_slot = np.full(S_pad, -1, dtype=np.int64)
    for t in range(T):
        o, S = tiles_off[t], tiles_S[t]
        tile_of_slot[o:o + S] = t
        lidx_of_slot[o:o + S] = np.arange(S)

    slot_of = np.full(N, -1, dtype=np.int64)
    for c in range(C):
        lo, hi = c * NPC, (c + 1) * NPC
        dn = D_node[lo:hi]
        for D in np.unique(dn):
            nodes = lo + np.nonzero(dn == D)[0]
            slots = bucket_slots[int(D)][: len(nodes)]
            slot_of[nodes] = slots
    assert (slot_of >= 0).all()

    node_of_slot = np.full((C, S_pad), -1, dtype=np.int64)
    node_of_slot[core_of, slot_of] = np.arange(N)

    order = np.argsort(dst, kind="stable")
    dst_s = dst[order]
    src_s = src[order]
    group_start = np.zeros(N + 1, dtype=np.int64)
    np.cumsum(np.bincount(dst_s, minlength=N), out=group_start[1:])
    rank = np.arange(E) - group_start[dst_s]
    e_core = core_of[dst_s]
    e_slot = slot_of[dst_s]
    e_tile = tile_of_slot[e_slot]
    e_part = lidx_of_slot[e_slot] * D_node[dst_s] + rank
    assert (e_part < P).all()

    ZROW1 = N
    idx_l1 = np.full((C, P, T), ZROW1, dtype=np.int32)
    idx_l1[e_core, e_part, e_tile] = src_s.astype(np.int32)

    ZROW2 = C * S_pad
    idx_l2 = np.full((C, P, T), ZROW2, dtype=np.int32)
    h_row = core_of[src_s] * S_pad + slot_of[src_s]
    idx_l2[e_core, e_part, e_tile] = h_row.astype(np.int32)

    masks = np.zeros((C, P, S_pad), dtype=np.float32)
    inv = (1.0 / np.maximum(deg, 1.0)).astype(np.float32)
    masks[e_core, e_part, e_slot] = inv[dst_s]

    NB = S_pad // P
    d1 = node_of_slot.reshape(C, NB, P).transpose(0, 2, 1).copy()
    idx_d1 = np.where(d1 >= 0, d1, ZROW1).astype(np.int32)

    pl = _Plan()
    pl.N, pl.E, pl.C, pl.NPC = N, E, C, NPC
    pl.S_pad, pl.T, pl.NB = S_pad, T, NB
    pl.tiles_D, pl.tiles_off, pl.tiles_S = tiles_D, tiles_off, tiles_S
    pl.idx_l1, pl.idx_l2, pl.idx_d1 = idx_l1, idx_l2, idx_d1
    pl.masks = masks
    pl.slot_of, pl.core_of = slot_of, core_of
    pl.deg = deg
    return pl


class Meta:
    def __init__(self, pl, K=K_CHUNK):
        self.N = pl.N
        self.C = pl.C
        self.S_pad = pl.S_pad
        self.T = pl.T
        self.NB = pl.NB
        self.NBANK = pl.S_pad // BANK
        self.K = K
        self.tiles = list(zip(pl.tiles_D.tolist(), pl.tiles_off.tolist(),
                              pl.tiles_S.tolist()))
        self.bank_tiles = [[] for _ in range(self.NBANK)]
        for t, (D, off, S) in enumerate(self.tiles):
            self.bank_tiles[off // BANK].append(t)
        self.bank_dead = []
        for b in range(self.NBANK):
            covered = np.zeros(BANK, bool)
            for t in self.bank_tiles[b]:
                D, off, S = self.tiles[t]
                covered[off - b * BANK: off - b * BANK + S] = True
            ranges = []
            i = 0
            while i < BANK:
                if not covered[i]:
                    j = i
                    while j < BANK and not covered[j]:
                        j += 1
                    ranges.append((i, j - i))
                    i = j
                else:
                    i += 1
            self.bank_dead.append(ranges)


def build_nc(meta):
    import concourse.bacc as bacc
    import concourse.mybir as mybir
    import concourse.tile as tile
    from concourse.bass import IndirectOffsetOnAxis

    F32 = mybir.dt.float32
    I32 = mybir.dt.int32

    nc = bacc.Bacc("TRN2", target_bir_lowering=False, debug=False,
                   num_devices=meta.C)
    N, S_pad, T, NB, NBANK, K = (meta.N, meta.S_pad, meta.T, meta.NB,
                                 meta.NBANK, meta.K)
    CS = meta.C * S_pad

    x_aug = nc.dram_tensor("x_aug", [N + 1, IN_C], F32, kind="ExternalInput")
    idx_l1 = nc.dram_tensor("idx_l1", [P, T], I32, kind="ExternalInput")
    idx_l2 = nc.dram_tensor("idx_l2", [P, T], I32, kind="ExternalInput")
    idx_d1 = nc.dram_tensor("idx_d1", [P, NB], I32, kind="ExternalInput")
    masks = nc.dram_tensor("masks", [P, S_pad], F32, kind="ExternalInput")
    Wl1 = nc.dram_tensor("Wl1", [IN_C, HID], F32, kind="ExternalInput")
    Wr1 = nc.dram_tensor("Wr1", [IN_C, HID], F32, kind="ExternalInput")
    Wl2 = nc.dram_tensor("Wl2", [HID, HID], F32, kind="ExternalInput")
    Wr2 = nc.dram_tensor("Wr2", [HID, HID], F32, kind="ExternalInput")
    bias1 = nc.dram_tensor("bias1", [HID, 1], F32, kind="ExternalInput")
    bias2 = nc.dram_tensor("bias2", [HID, 1], F32, kind="ExternalInput")
    ident = nc.dram_tensor("ident", [P, P], F32, kind="ExternalInput")
    out_d = nc.dram_tensor("out", [S_pad, HID], F32, kind="ExternalOutput")

    h_own = nc.dram_tensor("h_own", [S_pad, HID], F32)
    h_table = nc.dram_tensor("h_table", [CS + P, HID], F32, addr_space="Shared")

    with tile.TileContext(nc) as tc:
        with (
            tc.tile_pool(name="persist", bufs=1) as pp,
            tc.tile_pool(name="gather", bufs=2) as gp,
            tc.tile_pool(name="small", bufs=2) as sp,
            tc.tile_pool(name="psum", bufs=2, space="PSUM") as psp,
        ):
            idx1_sb = pp.tile([P, T], I32, tag="idx1")
            idx2_sb = pp.tile([P, T], I32, tag="idx2")
            idxd1_sb = pp.tile([P, NB], I32, tag="idxd1")
            wl1_sb = pp.tile([IN_C, HID], F32, tag="wl1")
            wr1_sb = pp.tile([IN_C, HID], F32, tag="wr1")
            wl2_sb = pp.tile([HID, HID], F32, tag="wl2")
            wr2_sb = pp.tile([HID, HID], F32, tag="wr2")
            b1_sb = pp.tile([HID, 1], F32, tag="b1")
            b2_sb = pp.tile([HID, 1], F32, tag="b2")
            id_sb = pp.tile([P, P], F32, tag="ident")
            hT_full = pp.tile([HID, S_pad], F32, tag="hT")
            h_stage = pp.tile([P, NB * HID], F32, tag="hstage")
            zero_sb = pp.tile([P, HID], F32, tag="zero")

            nc.sync.dma_start(out=idx1_sb[:], in_=idx_l1[:])
            nc.sync.dma_start(out=idx2_sb[:], in_=idx_l2[:])
            nc.sync.dma_start(out=idxd1_sb[:], in_=idx_d1[:])
            nc.sync.dma_start(out=wl1_sb[:], in_=Wl1[:])
            nc.sync.dma_start(out=wr1_sb[:], in_=Wr1[:])
            nc.sync.dma_start(out=wl2_sb[:], in_=Wl2[:])
            nc.sync.dma_start(out=wr2_sb[:], in_=Wr2[:])
            nc.sync.dma_start(out=b1_sb[:], in_=bias1[:])
            nc.sync.dma_start(out=b2_sb[:], in_=bias2[:])
            nc.sync.dma_start(out=id_sb[:], in_=ident[:])
            nc.vector.memset(zero_sb[:], 0.0)
            nc.sync.dma_start(out=h_table[CS:CS + P, :], in_=zero_sb[:])

            def layer(layer_i):
                first = layer_i == 0
                F = IN_C if first else HID
                idx_sb = idx1_sb if first else idx2_sb
                table = x_aug if first else h_table
                wl_sb, wr_sb = (wl1_sb, wr1_sb) if first else (wl2_sb, wr2_sb)
                b_sb = b1_sb if first else b2_sb
                stage = h_stage

                gbufs = {}

                def get_gbuf(c):
                    if c not in gbufs:
                        nk = min(K, T - c * K)
                        g = gp.tile([P, K * F], F32, tag=f"gbuf{layer_i}")
                        nc.gpsimd.indirect_dma_start(
                            out=g[:, :nk * F],
                            out_offset=None,
                            in_=table[:],
                            in_offset=IndirectOffsetOnAxis(
                                ap=idx_sb[:, c * K:c * K + nk], axis=0),
                        )
                        gbufs[c] = g
                    return gbufs[c]

                for b in range(NBANK):
                    mask_sb = sp.tile([P, BANK], F32, tag="mask")
                    nc.sync.dma_start(out=mask_sb[:],
                                      in_=masks[:, b * BANK:(b + 1) * BANK])
                    ps_seg = psp.tile([P, BANK], F32, tag="seg")
                    for (o, ln) in meta.bank_dead[b]:
                        nc.vector.memset(ps_seg[:F, o:o + ln], 0.0)
                    for t in meta.bank_tiles[b]:
                        D, off, S = meta.tiles[t]
                        o = off - b * BANK
                        g = get_gbuf(t // K)
                        j = t % K
                        nc.tensor.matmul(
                            out=ps_seg[:F, o:o + S],
                            lhsT=g[:, j * F:(j + 1) * F],
                            rhs=mask_sb[:, o:o + S],
                            start=True, stop=True,
                        )
                    aggT = sp.tile([P, BANK], F32, tag="aggT")
                    nc.vector.tensor_copy(out=aggT[:F], in_=ps_seg[:F])

                    if first:
                        ownT = sp.tile([P, BANK], F32, tag="ownT")
                        og = sp.tile([P, 4 * IN_C], F32, tag="og")
                        nc.gpsimd.indirect_dma_start(
                            out=og[:],
                            out_offset=None,
                            in_=table[:],
                            in_offset=IndirectOffsetOnAxis(
                                ap=idxd1_sb[:, b * 4:(b + 1) * 4], axis=0),
                        )
                        for blk in range(4):
                            ps_t = psp.tile([P, P], F32, tag="tp")
                            nc.tensor.transpose(
                                out=ps_t[:IN_C, :P],
                                in_=og[:, blk * IN_C:(blk + 1) * IN_C],
                                identity=id_sb[:],
                            )
                            nc.vector.tensor_copy(
                                out=ownT[:F, blk * P:(blk + 1) * P],
                                in_=ps_t[:F, :P])
                        own_rhs = ownT[:F, :]
                    else:
                        own_rhs = hT_full[:HID, b * BANK:(b + 1) * BANK]

                    ps_d = psp.tile([HID, BANK], F32, tag="down")
                    nc.tensor.matmul(out=ps_d[:], lhsT=wl_sb[:F, :],
                                     rhs=aggT[:F, :], start=True, stop=False)
                    nc.tensor.matmul(out=ps_d[:], lhsT=wr_sb[:F, :],
                                     rhs=own_rhs, start=False, stop=True)

                    hT_bank = sp.tile([HID, BANK], F32, tag="hTb")
                    if first:
                        nc.vector.tensor_scalar(
                            out=hT_bank[:], in0=ps_d[:], scalar1=b_sb[:, :1],
                            scalar2=0.0, op0=mybir.AluOpType.add,
                            op1=mybir.AluOpType.max)
                        nc.vector.tensor_copy(
                            out=hT_full[:, b * BANK:(b + 1) * BANK],
                            in_=hT_bank[:])
                    else:
                        nc.vector.tensor_scalar(
                            out=hT_bank[:], in0=ps_d[:], scalar1=b_sb[:, :1],
                            scalar2=None, op0=mybir.AluOpType.add)

                    for blk in range(4):
                        B = b * 4 + blk
                        ps_t2 = psp.tile([P, P], F32, tag="tp")
                        nc.tensor.transpose(
                            out=ps_t2[:P, :HID],
                            in_=hT_bank[:, blk * P:(blk + 1) * P],
                            identity=id_sb[:HID, :HID],
                        )
                        nc.vector.tensor_copy(
                            out=stage[:, B * HID:(B + 1) * HID],
                            in_=ps_t2[:P, :HID])

                dst = (h_own if first else out_d)
                nc.sync.dma_start(
                    out=dst[:].rearrange("(nb p) f -> p nb f", p=P),
                    in_=stage[:].rearrange("p (nb f) -> p nb f", f=HID),
                )
                if first:
                    nc.gpsimd.collective_compute(
                        "AllGather",
                        mybir.AluOpType.bypass,
                        replica_groups=[list(range(meta.C))],
                        ins=[h_own[:]],
                        outs=[h_table[0:CS, :]],
                    )

            layer(0)
            layer(1)

    nc.compile()
    return nc


def _in_maps(pl, x, W_l1, b_l1, W_r1, W_l2, b_l2, W_r2):
    x_aug = np.concatenate([x, np.zeros((1, IN_C), np.float32)], 0)
    x_aug = np.ascontiguousarray(x_aug)
    ident = np.eye(P, dtype=np.float32)
    maps = []
    for c in range(pl.C):
        maps.append({
            "x_aug": x_aug,
            "idx_l1": np.ascontiguousarray(pl.idx_l1[c]),
            "idx_l2": np.ascontiguousarray(pl.idx_l2[c]),
            "idx_d1": np.ascontiguousarray(pl.idx_d1[c]),
            "masks": np.ascontiguousarray(pl.masks[c]),
            "Wl1": np.ascontiguousarray(W_l1.astype(np.float32)),
            "Wr1": np.ascontiguousarray(W_r1.astype(np.float32)),
            "Wl2": np.ascontiguousarray(W_l2.astype(np.float32)),
            "Wr2": np.ascontiguousarray(W_r2.astype(np.float32)),
            "bias1": np.ascontiguousarray(
                np.asarray(b_l1, np.float32).reshape(HID, 1)),
            "bias2": np.ascontiguousarray(
                np.asarray(b_l2, np.float32).reshape(HID, 1)),
            "ident": ident,
        })
    return maps


_CACHE = {}


def kernel(x, edge_index, W_l1, b_l1, W_r1, W_l2, b_l2, W_r2,
           trace=False, _return_results=False):
    x = np.asarray(x, dtype=np.float32)
    edge_index = np.asarray(edge_index)
    N = x.shape[0]

    pl = build_plan(edge_index, N, C)
    meta = Meta(pl, K=K_CHUNK)

    # compile cache keyed by the compile-time structure
    key = (N, pl.S_pad, pl.T, tuple(pl.tiles_D.tolist()),
           tuple(pl.tiles_off.tolist()), tuple(pl.tiles_S.tolist()))
    if key not in _CACHE:
        _CACHE.clear()
        _CACHE[key] = build_nc(meta)
    nc = _CACHE[key]

    maps = _in_maps(pl, x, W_l1, b_l1, W_r1, W_l2, b_l2, W_r2)

    from concourse.bass_utils import run_bass_kernel_spmd
    res = run_bass_kernel_spmd(nc, maps, core_ids=list(range(C)),
                               trace=trace, trace_cores=[0] if trace else None)

    out = np.zeros((N, HID), np.float32)
    for c in range(C):
        r = res.results[c]["out"]
        nodes = np.nonzero(pl.core_of == c)[0]
        out[nodes] = r[pl.slot_of[nodes]]
    if _return_results:
        return out, res
    return out
